# revision 5
# baseline (speedup 1.0000x reference)
"""Trainium2 kernel for nn_MDA_MOE (MoE of 4 GNN experts x 3 branches).

Strategy: expert-parallel across 8 NeuronCores. Each of the 12 expert units
(3 branches x 4 expert types) is jit-compiled as its own small program and
dispatched asynchronously to an assigned device; JAX async dispatch runs
disjoint-device programs concurrently.

Aggregation (segment_sum / segment_max over edges) is reformulated as dense
count-matrix products (A[dst,src] = edge multiplicity, built on host from the
integer edge lists - pure index preprocessing). This is mathematically
identical to the reference segment ops (duplicates and self-loops included)
and avoids sparse scatter lowering on the device.

Matmuls run in bf16 with f32 accumulation; elementwise math stays f32.
"""

import jax
import jax.numpy as jnp
import numpy as np

N_M, N_D = 2048, 2048
IN_F, OUT_F = 1024, 512
NT = N_M + N_D
N_SAMP = 8192

BF = jnp.bfloat16


def mm(a, b):
    return jnp.matmul(a.astype(BF), b.astype(BF),
                      preferred_element_type=jnp.float32)


def lin(p, x):
    return mm(x, p["W"]) + p["b"]


elu = jax.nn.elu


def _degs(A):
    deg_in = jnp.maximum(A.sum(axis=1), 1.0)
    deg_out = jnp.maximum(A.sum(axis=0), 1.0)
    return deg_in, deg_out


def gcn_expert_d(p, x, A):
    deg_in, deg_out = _degs(A)
    ci, co = deg_out ** -0.5, deg_in ** -0.5
    res = elu(lin(p["res"], x))
    h = x
    for i, lp in enumerate(p["layers"]):
        h = mm(A, mm(h * ci[:, None], lp["W"])) * co[:, None] + lp["b"]
        if i < 2:
            h = elu(h)
    return h, res


def gin_expert_d(p, x, A):
    res = elu(lin(p["res"], x))
    h = x
    for i, mlp_p in enumerate(p["mlps"]):
        t = mm(h, mlp_p[0]["W"])
        h = elu(t + mm(A, t) + mlp_p[0]["b"])
        h = elu(lin(mlp_p[1], h))
        if i < 2:
            h = elu(h)
    return h, res


def sage_expert_d(p, x, A):
    deg_in, _ = _degs(A)
    res = elu(lin(p["res"], x))
    h = x
    for lp in p["layers"]:
        t = mm(h, lp["nb"]["W"])
        agg = mm(A, t) / deg_in[:, None]
        h = elu(mm(h, lp["sf"]) + agg + lp["nb"]["b"])
    return h, res


def gat_expert_d(p, x, A):
    res = elu(lin(p["res"], x))
    mask_neg = jnp.where(A > 0.0, 0.0, -1e30)  # [d, s]
    h = x
    for i, lp in enumerate(p["layers"]):
        z = mm(h, lp["W"])
        el = mm(z, lp["al"][:, None])[:, 0]
        er = mm(z, lp["ar"][:, None])[:, 0]
        e = jax.nn.leaky_relu(el[None, :] + er[:, None], 0.2) + mask_neg
        m = jnp.max(e, axis=1, keepdims=True)
        ex = jnp.exp(e - m) * A
        denom = ex.sum(axis=1, keepdims=True)
        h = mm(ex / denom, z) + lp["b"]
        if i < 2:
            h = elu(h)
    return h, res


EXPERT_FNS = {"gcn": gcn_expert_d, "gin": gin_expert_d,
              "sage": sage_expert_d, "gat": gat_expert_d}
KINDS = ["gcn", "gin", "sage", "gat"]

_jit_cache = {}


def _unit_jit(kind):
    if kind not in _jit_cache:
        fn = EXPERT_FNS[kind]
        _jit_cache[kind] = jax.jit(
            lambda p, x, A, fn=fn: fn(p, x, A))
    return _jit_cache[kind]


def _proj_jit():
    if "proj" not in _jit_cache:
        def proj(mi, di, wm, wd):
            return jnp.concatenate([mm(mi, wm), mm(di, wd)], axis=0)
        _jit_cache["proj"] = jax.jit(proj)
    return _jit_cache["proj"]


def _gate_jit():
    if "gate" not in _jit_cache:
        def gate_stage(gp, x, outs, ress, f_b, f_a):
            base = jax.nn.softmax(lin(gp["l2"], elu(lin(gp["l1"], x))), axis=1)
            var = jnp.var(outs, axis=2, ddof=1)
            ent = -jnp.sum(outs * jnp.log(jnp.maximum(outs, 0.0) + 1e-10),
                           axis=2)
            w = jax.nn.softmax(base * gp["perf"] * var * ent / gp["temp"],
                               axis=1)[:, :, None]
            emb = (outs * w).sum(axis=1)
            res = (ress * w).sum(axis=1)
            return f_b * emb + (1.0 - f_b) * res
        _jit_cache["gate"] = jax.jit(gate_stage)
    return _jit_cache["gate"]


def _final_jit():
    if "final" not in _jit_cache:
        def final(emb, mlp0, mlp1):
            h = elu(lin(mlp0, emb))
            result = jax.nn.sigmoid(lin(mlp1, h))
            return result
        _jit_cache["final"] = jax.jit(final)
    return _jit_cache["final"]


def kernel(params, miRNA, disease, mm_src, mm_dst, dd_src, dd_dst,
           md_src, md_dst, samples):
    devices = jax.devices()[:8]
    params = jax.tree_util.tree_map(np.asarray, params)
    miRNA = np.asarray(miRNA, np.float32)
    disease = np.asarray(disease, np.float32)
    samples_np = np.asarray(samples, np.int32)

    def build_A(src, dst, n):
        A = np.zeros((n, n), np.float32)
        np.add.at(A, (np.asarray(dst, np.int64), np.asarray(src, np.int64)),
                  1.0)
        return A

    A_m = build_A(mm_src, mm_dst, N_M)
    A_d = build_A(dd_src, dd_dst, N_D)
    A_a = build_A(md_src, md_dst, NT)

    dput = jax.device_put

    # ---- Stage A0: md projection on device 0 ----
    md_x = _proj_jit()(
        dput(miRNA, devices[0]), dput(disease, devices[0]),
        dput(params["lin_m"], devices[0]), dput(params["lin_d"], devices[0]))
    md_x.block_until_ready()

    # ---- Stage A: 12 expert units, expert-parallel ----
    # device c (0-3): m.KINDS[c] then d.KINDS[c]; device c+4: md.KINDS[c].
    # Dispatch md units first (largest), then m, then d.
    futures = {}
    for c, kind in enumerate(KINDS):
        dev = devices[c + 4]
        p = jax.tree_util.tree_map(lambda a: dput(a, dev),
                                   params["md"]["experts"][kind])
        futures[("md", kind)] = _unit_jit(kind)(
            p, dput(md_x, dev), dput(A_a, dev))
    for c, kind in enumerate(KINDS):
        dev = devices[c]
        p = jax.tree_util.tree_map(lambda a: dput(a, dev),
                                   params["m"]["experts"][kind])
        futures[("m", kind)] = _unit_jit(kind)(
            p, dput(miRNA, dev), dput(A_m, dev))
    for c, kind in enumerate(KINDS):
        dev = devices[c]
        p = jax.tree_util.tree_map(lambda a: dput(a, dev),
                                   params["d"]["experts"][kind])
        futures[("d", kind)] = _unit_jit(kind)(
            p, dput(disease, dev), dput(A_d, dev))

    outs = {}
    for key, (o, r) in futures.items():
        outs[key] = (np.asarray(o), np.asarray(r))

    # ---- Stage B: gates + fuse (3 branches on 3 devices, async) ----
    gate = _gate_jit()
    fw = {k: float(np.asarray(v)[0]) for k, v in params["fuse"].items()}

    def run_gate(branch, x, dev_i):
        dev = devices[dev_i]
        gp = jax.tree_util.tree_map(lambda a: dput(a, dev),
                                    params[branch]["gate"])
        o_stack = np.stack([outs[(branch, k)][0] for k in KINDS], axis=1)
        r_stack = np.stack([outs[(branch, k)][1] for k in KINDS], axis=1)
        fb = fw["m"] if branch == "m" else (
            fw["d"] if branch == "d" else fw["md"])
        return gate(gp, dput(x, dev), dput(o_stack, dev), dput(r_stack, dev),
                    fb, 0.0)

    emb_m_f = run_gate("m", miRNA, 0)
    emb_d_f = run_gate("d", disease, 1)
    emb_a_f = run_gate("md", np.asarray(md_x), 2)

    emb_m_np = np.asarray(emb_m_f)
    emb_d_np = np.asarray(emb_d_f)
    emb_a_np = np.asarray(emb_a_f)

    # ---- Stage C: fuse halves (device) + host sample gather + final MLP ----
    if "fuse2" not in _jit_cache:
        _jit_cache["fuse2"] = jax.jit(
            lambda a, b, w: w * a + (1.0 - w) * b)
    fuse2 = _jit_cache["fuse2"]
    dev = devices[0]
    emb_mm = np.asarray(fuse2(dput(emb_m_np, dev), dput(emb_a_np[:N_M], dev),
                              fw["w1"]))
    emb_dd = np.asarray(fuse2(dput(emb_d_np, dev), dput(emb_a_np[N_M:], dev),
                              fw["w2"]))
    emb = np.concatenate([emb_mm[samples_np[:, 0]],
                          emb_dd[samples_np[:, 1]]], axis=1)
    dev = devices[0]
    result = _final_jit()(
        dput(emb, dev),
        jax.tree_util.tree_map(lambda a: dput(a, dev), params["mlp"][0]),
        jax.tree_util.tree_map(lambda a: dput(a, dev), params["mlp"][1]))

    return (np.asarray(result), emb_m_np, emb_a_np[:N_M],
            emb_d_np, emb_a_np[N_M:], np.asarray(emb))
